# revision 22
# baseline (speedup 1.0000x reference)
"""Multi-head self-attention (B=4, S=2048, D=1024, H=16, Hd=64) on 8 TRN2 cores.

Sharding: core c -> (batch b = c//2, head-group g = c%2 of 8 heads).
v2: per-head-pair projection/attention software pipelining (projections of
pair t+1 fill PE idle while pair t's exp stream runs on the Scalar engine),
fp8e4 DoubleRow PV matmuls (exp slabs and V quantized to fp8), and a fused
divide for the softmax normalization (denominator via ones-column in V,
bounce-broadcast through DRAM).
"""

from contextlib import ExitStack

import numpy as np
import ml_dtypes

import concourse.bass as bass
import concourse.tile as tile
from concourse import mybir
from concourse.bass_utils import run_bass_kernel_spmd
from concourse.vector_clock import ScopedClock
from bass_rust import InstNoOp, SyncInfo

BF16 = mybir.dt.bfloat16
F32 = mybir.dt.float32
FP8 = mybir.dt.float8e4
AF = mybir.ActivationFunctionType
DR = mybir.MatmulPerfMode.DoubleRow

B, S, D = 4, 2048, 1024
H, HD = 16, 64
GH = 8          # heads per core (head-group size)
GM = GH * HD    # 512 head dims per core
NQB = 4         # q blocks of 512
QB = 512
NKC = 16        # k chunks of 128
NKP = 8         # k chunk pairs (DoubleRow granularity)
NDC = 8         # d chunks of 128 (contraction for projections)
# per-(pair-head, kcpair) fp8 V block: 2 slots x 128 cols
# (64 data + ones col at 64 + 63 zero pad; DoubleRow needs M=128)
VB = 2 * 128

_META_TYPES = ("TileBranchInst", "BassTileLoopBlock", "BassTilePoolBoundary")


class _TileCtx(tile.TileContext):
    """Splits multi-sem-wait instructions: the pinned walrus rejects any TPB
    instruction carrying more than one sem-wait, while Tile emits joins and a
    global end-of-context drain with several."""

    def _split_waits(self, ordered):
        nc = self.nc
        for bb_name, insts in ordered.items():
            out = []
            for inst in insts:
                si = inst.sync_info
                if (
                    si is not None
                    and si.on_wait
                    and len(si.on_wait) > 1
                    and type(inst).__name__ not in _META_TYPES
                    and inst.engine != mybir.EngineType.Unassigned
                ):
                    waits = list(si.on_wait)
                    for w in waits[:-1]:
                        nop = InstNoOp(
                            name=nc.get_next_instruction_name(), ins=[], outs=[]
                        )
                        nop.engine = inst.engine
                        nop.sync_info = SyncInfo(on_wait=[w], on_update=[])
                        out.append(nop)
                    inst.sync_info = SyncInfo(
                        on_wait=[waits[-1]], on_update=list(si.on_update)
                    )
                out.append(inst)
            ordered[bb_name] = out

    def _lower_ordered_insts(self, ordered):
        self._split_waits(ordered)
        super()._lower_ordered_insts(ordered)

    def _drain_and_barrier(self, tick_clock, wait_clock):
        drain_inst = self.nc.sync.drain()
        wait_clock.add_sem_waits(
            drain_inst.ins, ScopedClock({None: tick_clock.global_clock})
        )
        si = drain_inst.ins.sync_info
        waits = list(si.on_wait) if si is not None else []
        if len(waits) > 1:
            drain_inst.ins.sync_info = SyncInfo(
                on_wait=waits[:1], on_update=list(si.on_update)
            )
            for w in waits[1:]:
                extra = self.nc.sync.drain()
                extra.ins.sync_info = SyncInfo(on_wait=[w], on_update=[])

        self.nc.all_engine_barrier()
        assert self.sems is not None
        popped = self.nc._tile_sem_poison_stack.pop()
        assert popped is self._sem_poison
        self.nc.clear_and_free_semaphores(list(self.sems.allocated().values()))
        self.nc.all_engine_barrier()


def _build_program():
    nc = bass.Bass(trn_type="TRN2", debug=False, num_devices=8)

    xT = nc.dram_tensor("xT", [D, S], BF16, kind="ExternalInput").ap()
    wq = nc.dram_tensor("wq", [D, GM], BF16, kind="ExternalInput").ap()
    wk = nc.dram_tensor("wk", [D, GM], BF16, kind="ExternalInput").ap()
    wv = nc.dram_tensor("wv", [D, GM], BF16, kind="ExternalInput").ap()
    # pair-major-reordered Wo.T slice: [128, 4 pairs x 1024]
    wo = nc.dram_tensor("wo", [128, (GM // 128) * D], BF16, kind="ExternalInput").ap()
    bq = nc.dram_tensor("bq", [GM], F32, kind="ExternalInput").ap()
    bk = nc.dram_tensor("bk", [GM], F32, kind="ExternalInput").ap()
    bo = nc.dram_tensor("bo", [D], F32, kind="ExternalInput").ap()
    outT = nc.dram_tensor("outT", [D, S], F32, kind="ExternalOutput").ap()

    with _TileCtx(nc) as tc, ExitStack() as ctx:
        const_pool = ctx.enter_context(tc.tile_pool(name="const", bufs=1))
        act_pool = ctx.enter_context(tc.tile_pool(name="acts", bufs=1))
        w_pool = ctx.enter_context(tc.tile_pool(name="wts", bufs=1))

        # ---- constants / weights / inputs -------------------------------
        bq_sb = const_pool.tile([128, NDC // 2], F32, tag="bq")
        nc.sync.dma_start(bq_sb[:], bq.rearrange("(c p) -> p c", p=128))
        bk_sb = const_pool.tile([128, NDC // 2], F32, tag="bk")
        nc.sync.dma_start(bk_sb[:], bk.rearrange("(c p) -> p c", p=128))
        bo_sb = const_pool.tile([128, NDC], F32, tag="bo")
        nc.sync.dma_start(bo_sb[:], bo.rearrange("(c p) -> p c", p=128))

        # input + QK/V weights first (gate the first projections); wo last
        # (only needed by the final output projection).
        xt = w_pool.tile([128, NDC * S], BF16, tag="xt")
        for t in range(NDC):
            nc.sync.dma_start(
                xt[:, t * S : (t + 1) * S], xT[t * 128 : (t + 1) * 128, :]
            )
        # per-pair column-group DMAs, pair 0 first, so pair-0 projections
        # start as soon as x plus one pair's weights have landed.
        wq_sb = w_pool.tile([128, NDC * GM], BF16, tag="wq")
        wk_sb = w_pool.tile([128, NDC * GM], BF16, tag="wk")
        wv_sb = w_pool.tile([128, NDC * GM], BF16, tag="wv")
        for p in range(4):
            msl = slice(p * 128, (p + 1) * 128)
            for w_sb, src in ((wq_sb, wq), (wk_sb, wk), (wv_sb, wv)):
                nc.sync.dma_start(
                    w_sb[:].rearrange("p (c m) -> p c m", m=GM)[:, :, msl],
                    src.rearrange("(c p) m -> p c m", p=128)[:, :, msl],
                )
        wo_sb = const_pool.tile([128, (GM // 128) * D], BF16, tag="wo")
        nc.sync.dma_start(wo_sb[:], wo[:, :])

        # activations
        qt = act_pool.tile([128, (GM // 128) * S], BF16, tag="qt")
        kt = act_pool.tile([128, (GM // 128) * S], BF16, tag="kt")
        # V in fp8, DoubleRow layout: per (pair t, kcpair kp, pair-head) a
        # block of 2 slots x 128 cols; col 64 of each slot is 1.0 (softmax
        # denominator lands in PSUM row 64), cols 65-127 stay zero.
        v_sb = act_pool.tile([128, 4 * NKP * 2 * VB], FP8, tag="v")
        nc.vector.memset(v_sb[:], 0.0)
        nc.vector.memset(
            v_sb[:].rearrange("p (b sl c) -> p b sl c", sl=2, c=128)[:, :, :, 64:65],
            1.0,
        )
        # attention output (normalized), transposed per pair: [128, S]
        otp = [
            act_pool.tile([128, S], BF16, name=f"otp{t}", tag=f"otp{t}")
            for t in range(GH // 2)
        ]

        with tc.tile_pool(name="qk_psum", bufs=2, space="PSUM") as qk_psum, \
             tc.tile_pool(name="v_psum", bufs=1, space="PSUM") as v_psum, \
             tc.tile_pool(name="s_psum", bufs=2, space="PSUM") as s_psum, \
             tc.tile_pool(name="o_psum", bufs=1, space="PSUM") as o_psum, \
             tc.tile_pool(name="dscr", bufs=4, space="DRAM") as dram_pool, \
             tc.tile_pool(name="slab", bufs=40) as slab_pool, \
             tc.tile_pool(name="norm", bufs=2) as norm_pool, \
             tc.tile_pool(name="pnorm", bufs=2) as pn_pool, \
             tc.tile_pool(name="shift", bufs=2) as shift_pool:
            # PV consumption runs one (t, qb) iteration behind slab
            # production so its semaphore waits are already satisfied when
            # the PE pops them (avoids the per-matmul SBUF-access-latency
            # exposure on fresh waits).
            pending = []  # deferred closures: PV+stash for the previous iter
            for t in range(GH // 2):  # head pairs (2t, 2t+1)
                # ---- projections for pair t (overlap prior pair's attn) --
                for w_sb, b_sb, dst in ((wq_sb, bq_sb, qt), (wk_sb, bk_sb, kt)):
                    for sh in range(4):  # S quarters
                        ps = qk_psum.tile([128, QB], F32, tag="qkp")
                        for dc in range(NDC):
                            nc.tensor.matmul(
                                ps[:],
                                w_sb[:, dc * GM + t * 128 : dc * GM + (t + 1) * 128],
                                xt[:, dc * S + sh * QB : dc * S + (sh + 1) * QB],
                                start=(dc == 0),
                                stop=(dc == NDC - 1),
                            )
                        nc.vector.tensor_scalar_add(
                            dst[:, t * S + sh * QB : t * S + (sh + 1) * QB],
                            ps[:],
                            b_sb[:, t : t + 1],
                        )
                for sg in range(4):  # groups of 4 k-chunks
                    psv = v_psum.tile([128, 512], F32, tag="vp")
                    for sl4 in range(4):
                        si = sg * 4 + sl4
                        for dc in range(NDC):
                            nc.tensor.matmul(
                                psv[:, sl4 * 128 : (sl4 + 1) * 128],
                                xt[:, dc * S + si * 128 : dc * S + (si + 1) * 128],
                                wv_sb[:, dc * GM + t * 128 : dc * GM + (t + 1) * 128],
                                start=(dc == 0),
                                stop=(dc == NDC - 1),
                            )
                    # quantize to fp8 into the DoubleRow layout; data cols
                    # only (ones/zero columns pre-set by memset).
                    for kpl in range(2):
                        kp = 2 * sg + kpl
                        base = (t * NKP + kp) * 2 * VB
                        dst = (
                            v_sb[:, base : base + 2 * VB]
                            .rearrange("p (hh sl c) -> p sl hh c", hh=2, sl=2)[
                                :, :, :, 0:HD
                            ]
                        )
                        src = psv[
                            :, kpl * 256 : (kpl + 1) * 256
                        ].rearrange("p (sl hh c) -> p sl hh c", sl=2, hh=2)
                        nc.vector.tensor_copy(dst, src)

                # ---- attention for pair t --------------------------------
                # Softmax denominators for the whole pair are gathered into
                # one [8, 512] tile (row 2*qb+i) so a single batched
                # reciprocal serves all 8 (head, qb) groups: the DVE
                # reciprocal costs ~7 cycles per free-dim column regardless
                # of partition count.
                den_all = norm_pool.tile([GH, QB], F32, tag="den_all")
                pn = pn_pool.tile([128, S], BF16, tag="pn")
                for qb in range(NQB):
                    q01 = [
                        qt[i * 64 : (i + 1) * 64, t * S + qb * QB : t * S + (qb + 1) * QB]
                        for i in range(2)
                    ]
                    slabs = [[], []]
                    for kp in range(NKP):
                        for i in range(2):
                            with tc.high_priority(offset=300):
                                ps = s_psum.tile([128, 2 * QB], F32, tag="sp")
                                for j in range(2):
                                    kc = 2 * kp + j
                                    ksl = slice(
                                        t * S + kc * 128, t * S + (kc + 1) * 128
                                    )
                                    nc.tensor.matmul(
                                        ps[:, j * QB : (j + 1) * QB],
                                        kt[i * 64 : (i + 1) * 64, ksl],
                                        q01[i],
                                        start=True,
                                        stop=True,
                                    )
                                sl = slab_pool.tile([128, 2 * QB], FP8, tag="slab")
                                nc.scalar.activation(sl[:], ps[:], AF.Exp, scale=0.125)
                            slabs[i].append(sl)

                    def pv_stage(t=t, qb=qb, slabs=slabs, den_all=den_all, pn=pn):
                        for i in range(2):  # heads within the pair
                            po = o_psum.tile([128, QB], F32, tag="op")
                            for kp in range(NKP):
                                vblk = (
                                    v_sb[
                                        :,
                                        (t * NKP + kp) * 2 * VB
                                        + i * VB : (t * NKP + kp) * 2 * VB
                                        + (i + 1) * VB,
                                    ]
                                    .rearrange("p (sl c) -> p sl c", sl=2)
                                )
                                nc.tensor.matmul(
                                    po[:, :],
                                    vblk,
                                    slabs[i][kp][:].rearrange(
                                        "p (sl n) -> p sl n", sl=2
                                    ),
                                    start=(kp == 0),
                                    stop=(kp == NKP - 1),
                                    perf_mode=DR,
                                )
                            # stash raw PV output into pn and the denominator
                            # row into den_all; normalization batched per pair.
                            den = norm_pool.tile([128, QB], F32, tag="den")
                            nc.vector.tensor_copy(den[64:65, :], po[64:65, :])
                            nc.sync.dma_start(
                                den_all[2 * qb + i : 2 * qb + i + 1, :],
                                den[64:65, :],
                            )
                            if i == 0:
                                nc.vector.tensor_copy(
                                    pn[0:HD, qb * QB : (qb + 1) * QB], po[0:HD, :]
                                )
                            else:
                                tmp = shift_pool.tile([HD, QB], BF16, tag="tmp")
                                nc.vector.tensor_copy(tmp[:], po[0:HD, :])
                                nc.sync.dma_start(
                                    pn[HD:128, qb * QB : (qb + 1) * QB], tmp[:]
                                )

                    pending.append(pv_stage)
                    if len(pending) > 1:
                        pending.pop(0)()

                def norm_stage(t=t, den_all=den_all, pn=pn):
                    # ---- batched normalization for pair t ----------------
                    rec_all = norm_pool.tile([GH, QB], F32, tag="rec_all")
                    nc.vector.reciprocal(rec_all[:], den_all[:])
                    scr8 = dram_pool.tile([GH, QB], F32, tag="scr8")
                    nc.sync.dma_start(scr8, rec_all[:])
                    for qb in range(NQB):
                        for i in range(2):
                            r = 2 * qb + i
                            bcast = norm_pool.tile([128, QB], F32, tag="bcast")
                            rows = slice(i * HD, (i + 1) * HD)
                            nc.sync.dma_start(
                                bcast[rows, :],
                                scr8[r : r + 1, :].broadcast_to([HD, QB]),
                            )
                            nc.vector.tensor_mul(
                                otp[t][rows, qb * QB : (qb + 1) * QB],
                                pn[rows, qb * QB : (qb + 1) * QB],
                                bcast[rows, :],
                            )

                pending.append(norm_stage)
                if len(pending) > 1:
                    pending.pop(0)()
            for fn in pending:
                fn()

        # ---- output projection ------------------------------------------
        with tc.tile_pool(name="out_psum", bufs=2, space="PSUM") as out_psum, \
             tc.tile_pool(name="y", bufs=1) as y_pool:
            for ec in range(NDC):
                ps = out_psum.tile([128, S], F32, tag="yp")
                for mt in range(GM // 128):
                    lhsT = wo_sb[:, mt * D + ec * 128 : mt * D + (ec + 1) * 128]
                    for qb in range(NQB):
                        nc.tensor.matmul(
                            ps[:, qb * QB : (qb + 1) * QB],
                            lhsT,
                            otp[mt][:, qb * QB : (qb + 1) * QB],
                            start=(mt == 0),
                            stop=(mt == GM // 128 - 1),
                        )
                y_sb = y_pool.tile([128, S], F32, tag="y")
                nc.vector.tensor_scalar_add(y_sb[:], ps[:], bo_sb[:, ec : ec + 1])
                nc.sync.dma_start(outT[ec * 128 : (ec + 1) * 128, :], y_sb[:])

    return nc


_NC = None
_last_in_maps = None


def _get_program():
    global _NC
    if _NC is None:
        _NC = _build_program()
    return _NC


def build_in_maps(inputs):
    x = np.asarray(inputs["x"], np.float32)
    Wq, bq = inputs["Wq"], inputs["bq"]
    Wk, bk = inputs["Wk"], inputs["bk"]
    Wv, bv = inputs["Wv"], inputs["bv"]
    Wo, bo = inputs["Wo"], inputs["bo"]
    bf = ml_dtypes.bfloat16
    in_maps = []
    for c in range(8):
        b, g = c // 2, c % 2
        sl = slice(g * GM, (g + 1) * GM)
        wo_slice = np.asarray(Wo, np.float32)[:, sl].T  # [512, 1024]
        # fold bv and half of bo into the output bias
        bo_eff = np.asarray(bo, np.float32) / 2.0 + np.asarray(bv, np.float32)[sl] @ wo_slice
        in_maps.append(
            {
                "xT": np.ascontiguousarray(x[b].T).astype(bf),
                "wq": np.ascontiguousarray(np.asarray(Wq, np.float32)[sl, :].T).astype(bf),
                "wk": np.ascontiguousarray(np.asarray(Wk, np.float32)[sl, :].T).astype(bf),
                "wv": np.ascontiguousarray(np.asarray(Wv, np.float32)[sl, :].T).astype(bf),
                "wo": np.ascontiguousarray(
                    wo_slice.reshape(GM // 128, 128, D).transpose(1, 0, 2).reshape(128, (GM // 128) * D)
                ).astype(bf),
                "bq": np.ascontiguousarray(np.asarray(bq, np.float32)[sl]),
                "bk": np.ascontiguousarray(np.asarray(bk, np.float32)[sl]),
                "bo": np.ascontiguousarray(bo_eff.astype(np.float32)),
            }
        )
    return in_maps


def assemble_output(results):
    out = np.empty((B, S, D), np.float32)
    for b in range(B):
        acc = results[2 * b]["outT"].astype(np.float32) + results[2 * b + 1][
            "outT"
        ].astype(np.float32)
        out[b] = acc.T
    return out


def kernel(x, Wq, bq, Wk, bk, Wv, bv, Wo, bo):
    in_maps = build_in_maps(
        dict(x=x, Wq=Wq, bq=bq, Wk=Wk, bk=bk, Wv=Wv, bv=bv, Wo=Wo, bo=bo)
    )
    global _last_in_maps
    _last_in_maps = in_maps
    nc = _get_program()
    res = run_bass_kernel_spmd(nc, in_maps, core_ids=list(range(8)))
    return assemble_output(res.results)



# revision 23
# speedup vs baseline: 1.0566x; 1.0566x over previous
"""Multi-head self-attention (B=4, S=2048, D=1024, H=16, Hd=64) on 8 TRN2 cores.

Sharding: core c -> (batch b = c//2, head-group g = c%2 of 8 heads).
v2: per-head-pair projection/attention software pipelining (projections of
pair t+1 fill PE idle while pair t's exp stream runs on the Scalar engine),
fp8e4 DoubleRow PV matmuls (exp slabs and V quantized to fp8), and a fused
divide for the softmax normalization (denominator via ones-column in V,
bounce-broadcast through DRAM).
"""

from contextlib import ExitStack

import numpy as np
import ml_dtypes

import concourse.bass as bass
import concourse.tile as tile
from concourse import mybir
from concourse.bass_utils import run_bass_kernel_spmd
from concourse.vector_clock import ScopedClock
from bass_rust import InstNoOp, SyncInfo

BF16 = mybir.dt.bfloat16
F32 = mybir.dt.float32
FP8 = mybir.dt.float8e4
AF = mybir.ActivationFunctionType
DR = mybir.MatmulPerfMode.DoubleRow

B, S, D = 4, 2048, 1024
H, HD = 16, 64
GH = 8          # heads per core (head-group size)
GM = GH * HD    # 512 head dims per core
NQB = 4         # q blocks of 512
QB = 512
NKC = 16        # k chunks of 128
NKP = 8         # k chunk pairs (DoubleRow granularity)
NDC = 8         # d chunks of 128 (contraction for projections)
# per-(pair-head, kcpair) fp8 V block: 2 slots x 128 cols
# (64 data + ones col at 64 + 63 zero pad; DoubleRow needs M=128)
VB = 2 * 128

_META_TYPES = ("TileBranchInst", "BassTileLoopBlock", "BassTilePoolBoundary")


class _TileCtx(tile.TileContext):
    """Splits multi-sem-wait instructions: the pinned walrus rejects any TPB
    instruction carrying more than one sem-wait, while Tile emits joins and a
    global end-of-context drain with several."""

    def _split_waits(self, ordered):
        nc = self.nc
        for bb_name, insts in ordered.items():
            out = []
            for inst in insts:
                si = inst.sync_info
                if (
                    si is not None
                    and si.on_wait
                    and len(si.on_wait) > 1
                    and type(inst).__name__ not in _META_TYPES
                    and inst.engine != mybir.EngineType.Unassigned
                ):
                    waits = list(si.on_wait)
                    for w in waits[:-1]:
                        nop = InstNoOp(
                            name=nc.get_next_instruction_name(), ins=[], outs=[]
                        )
                        nop.engine = inst.engine
                        nop.sync_info = SyncInfo(on_wait=[w], on_update=[])
                        out.append(nop)
                    inst.sync_info = SyncInfo(
                        on_wait=[waits[-1]], on_update=list(si.on_update)
                    )
                out.append(inst)
            ordered[bb_name] = out

    def _lower_ordered_insts(self, ordered):
        self._split_waits(ordered)
        super()._lower_ordered_insts(ordered)

    def _drain_and_barrier(self, tick_clock, wait_clock):
        drain_inst = self.nc.sync.drain()
        wait_clock.add_sem_waits(
            drain_inst.ins, ScopedClock({None: tick_clock.global_clock})
        )
        si = drain_inst.ins.sync_info
        waits = list(si.on_wait) if si is not None else []
        if len(waits) > 1:
            drain_inst.ins.sync_info = SyncInfo(
                on_wait=waits[:1], on_update=list(si.on_update)
            )
            for w in waits[1:]:
                extra = self.nc.sync.drain()
                extra.ins.sync_info = SyncInfo(on_wait=[w], on_update=[])

        self.nc.all_engine_barrier()
        assert self.sems is not None
        popped = self.nc._tile_sem_poison_stack.pop()
        assert popped is self._sem_poison
        self.nc.clear_and_free_semaphores(list(self.sems.allocated().values()))
        self.nc.all_engine_barrier()


def _build_program():
    nc = bass.Bass(trn_type="TRN2", debug=False, num_devices=8)

    xT = nc.dram_tensor("xT", [D, S], BF16, kind="ExternalInput").ap()
    wq = nc.dram_tensor("wq", [D, GM], BF16, kind="ExternalInput").ap()
    wk = nc.dram_tensor("wk", [D, GM], BF16, kind="ExternalInput").ap()
    wv = nc.dram_tensor("wv", [D, GM], BF16, kind="ExternalInput").ap()
    # pair-major-reordered Wo.T slice: [128, 4 pairs x 1024]
    wo = nc.dram_tensor("wo", [128, (GM // 128) * D], BF16, kind="ExternalInput").ap()
    bq = nc.dram_tensor("bq", [GM], F32, kind="ExternalInput").ap()
    bk = nc.dram_tensor("bk", [GM], F32, kind="ExternalInput").ap()
    bo = nc.dram_tensor("bo", [D], F32, kind="ExternalInput").ap()
    outT = nc.dram_tensor("outT", [D, S], F32, kind="ExternalOutput").ap()

    with _TileCtx(nc) as tc, ExitStack() as ctx:
        const_pool = ctx.enter_context(tc.tile_pool(name="const", bufs=1))
        act_pool = ctx.enter_context(tc.tile_pool(name="acts", bufs=1))
        w_pool = ctx.enter_context(tc.tile_pool(name="wts", bufs=1))

        # ---- constants / weights / inputs -------------------------------
        bq_sb = const_pool.tile([128, NDC // 2], F32, tag="bq")
        nc.sync.dma_start(bq_sb[:], bq.rearrange("(c p) -> p c", p=128))
        bk_sb = const_pool.tile([128, NDC // 2], F32, tag="bk")
        nc.sync.dma_start(bk_sb[:], bk.rearrange("(c p) -> p c", p=128))
        bo_sb = const_pool.tile([128, NDC], F32, tag="bo")
        nc.sync.dma_start(bo_sb[:], bo.rearrange("(c p) -> p c", p=128))

        wo_sb = const_pool.tile([128, (GM // 128) * D], BF16, tag="wo")
        nc.sync.dma_start(wo_sb[:], wo[:, :])

        xt = w_pool.tile([128, NDC * S], BF16, tag="xt")
        for t in range(NDC):
            nc.sync.dma_start(
                xt[:, t * S : (t + 1) * S], xT[t * 128 : (t + 1) * 128, :]
            )
        wq_sb = w_pool.tile([128, NDC * GM], BF16, tag="wq")
        nc.sync.dma_start(
            wq_sb[:].rearrange("p (c m) -> p c m", m=GM),
            wq.rearrange("(c p) m -> p c m", p=128),
        )
        wk_sb = w_pool.tile([128, NDC * GM], BF16, tag="wk")
        nc.sync.dma_start(
            wk_sb[:].rearrange("p (c m) -> p c m", m=GM),
            wk.rearrange("(c p) m -> p c m", p=128),
        )
        wv_sb = w_pool.tile([128, NDC * GM], BF16, tag="wv")
        nc.sync.dma_start(
            wv_sb[:].rearrange("p (c m) -> p c m", m=GM),
            wv.rearrange("(c p) m -> p c m", p=128),
        )

        # activations
        qt = act_pool.tile([128, (GM // 128) * S], BF16, tag="qt")
        kt = act_pool.tile([128, (GM // 128) * S], BF16, tag="kt")
        # V in fp8, DoubleRow layout: per (pair t, kcpair kp, pair-head) a
        # block of 2 slots x 128 cols; col 64 of each slot is 1.0 (softmax
        # denominator lands in PSUM row 64), cols 65-127 stay zero.
        v_sb = act_pool.tile([128, 4 * NKP * 2 * VB], FP8, tag="v")
        nc.vector.memset(v_sb[:], 0.0)
        nc.vector.memset(
            v_sb[:].rearrange("p (b sl c) -> p b sl c", sl=2, c=128)[:, :, :, 64:65],
            1.0,
        )
        # attention output (normalized), transposed per pair: [128, S]
        otp = [
            act_pool.tile([128, S], BF16, name=f"otp{t}", tag=f"otp{t}")
            for t in range(GH // 2)
        ]

        with tc.tile_pool(name="qk_psum", bufs=2, space="PSUM") as qk_psum, \
             tc.tile_pool(name="v_psum", bufs=1, space="PSUM") as v_psum, \
             tc.tile_pool(name="s_psum", bufs=2, space="PSUM") as s_psum, \
             tc.tile_pool(name="o_psum", bufs=1, space="PSUM") as o_psum, \
             tc.tile_pool(name="dscr", bufs=4, space="DRAM") as dram_pool, \
             tc.tile_pool(name="slab", bufs=24) as slab_pool, \
             tc.tile_pool(name="norm", bufs=4) as norm_pool, \
             tc.tile_pool(name="pnorm", bufs=2) as pn_pool, \
             tc.tile_pool(name="shift", bufs=2) as shift_pool:
            for t in range(GH // 2):  # head pairs (2t, 2t+1)
                # ---- projections for pair t (overlap prior pair's attn) --
                for w_sb, b_sb, dst in ((wq_sb, bq_sb, qt), (wk_sb, bk_sb, kt)):
                    for sh in range(4):  # S quarters
                        ps = qk_psum.tile([128, QB], F32, tag="qkp")
                        for dc in range(NDC):
                            nc.tensor.matmul(
                                ps[:],
                                w_sb[:, dc * GM + t * 128 : dc * GM + (t + 1) * 128],
                                xt[:, dc * S + sh * QB : dc * S + (sh + 1) * QB],
                                start=(dc == 0),
                                stop=(dc == NDC - 1),
                            )
                        nc.vector.tensor_scalar_add(
                            dst[:, t * S + sh * QB : t * S + (sh + 1) * QB],
                            ps[:],
                            b_sb[:, t : t + 1],
                        )
                for sg in range(4):  # groups of 4 k-chunks
                    psv = v_psum.tile([128, 512], F32, tag="vp")
                    for sl4 in range(4):
                        si = sg * 4 + sl4
                        for dc in range(NDC):
                            nc.tensor.matmul(
                                psv[:, sl4 * 128 : (sl4 + 1) * 128],
                                xt[:, dc * S + si * 128 : dc * S + (si + 1) * 128],
                                wv_sb[:, dc * GM + t * 128 : dc * GM + (t + 1) * 128],
                                start=(dc == 0),
                                stop=(dc == NDC - 1),
                            )
                    # quantize to fp8 into the DoubleRow layout; data cols
                    # only (ones/zero columns pre-set by memset).
                    for kpl in range(2):
                        kp = 2 * sg + kpl
                        base = (t * NKP + kp) * 2 * VB
                        dst = (
                            v_sb[:, base : base + 2 * VB]
                            .rearrange("p (hh sl c) -> p sl hh c", hh=2, sl=2)[
                                :, :, :, 0:HD
                            ]
                        )
                        src = psv[
                            :, kpl * 256 : (kpl + 1) * 256
                        ].rearrange("p (sl hh c) -> p sl hh c", sl=2, hh=2)
                        nc.vector.tensor_copy(dst, src)

                # ---- attention for pair t --------------------------------
                # Softmax denominators for the whole pair are gathered into
                # one [8, 512] tile (row 2*qb+i) so a single batched
                # reciprocal serves all 8 (head, qb) groups: the DVE
                # reciprocal costs ~7 cycles per free-dim column regardless
                # of partition count.
                den_all = norm_pool.tile([GH, QB], F32, tag="den_all")
                pn = pn_pool.tile([128, S], BF16, tag="pn")
                for qb in range(NQB):
                    q01 = [
                        qt[i * 64 : (i + 1) * 64, t * S + qb * QB : t * S + (qb + 1) * QB]
                        for i in range(2)
                    ]
                    slabs = [[], []]
                    for kp in range(NKP):
                        for i in range(2):
                            with tc.high_priority(offset=300):
                                ps = s_psum.tile([128, 2 * QB], F32, tag="sp")
                                for j in range(2):
                                    kc = 2 * kp + j
                                    ksl = slice(
                                        t * S + kc * 128, t * S + (kc + 1) * 128
                                    )
                                    nc.tensor.matmul(
                                        ps[:, j * QB : (j + 1) * QB],
                                        kt[i * 64 : (i + 1) * 64, ksl],
                                        q01[i],
                                        start=True,
                                        stop=True,
                                    )
                                sl = slab_pool.tile([128, 2 * QB], FP8, tag="slab")
                                nc.scalar.activation(sl[:], ps[:], AF.Exp, scale=0.125)
                            slabs[i].append(sl)
                    for i in range(2):  # heads within the pair
                        po = o_psum.tile([128, QB], F32, tag="op")
                        for kp in range(NKP):
                            vblk = (
                                v_sb[
                                    :,
                                    (t * NKP + kp) * 2 * VB
                                    + i * VB : (t * NKP + kp) * 2 * VB
                                    + (i + 1) * VB,
                                ]
                                .rearrange("p (sl c) -> p sl c", sl=2)
                            )
                            nc.tensor.matmul(
                                po[:, :],
                                vblk,
                                slabs[i][kp][:].rearrange(
                                    "p (sl n) -> p sl n", sl=2
                                ),
                                start=(kp == 0),
                                stop=(kp == NKP - 1),
                                perf_mode=DR,
                            )
                        # stash the raw (un-normalized) PV output into otp and
                        # the denominator row into den_all; normalization is
                        # batched at end-of-pair.
                        den = norm_pool.tile([128, QB], F32, tag="den")
                        nc.vector.tensor_copy(den[64:65, :], po[64:65, :])
                        nc.sync.dma_start(
                            den_all[2 * qb + i : 2 * qb + i + 1, :], den[64:65, :]
                        )
                        if i == 0:
                            nc.vector.tensor_copy(
                                pn[0:HD, qb * QB : (qb + 1) * QB], po[0:HD, :]
                            )
                        else:
                            tmp = shift_pool.tile([HD, QB], BF16, tag="tmp")
                            nc.vector.tensor_copy(tmp[:], po[0:HD, :])
                            nc.sync.dma_start(
                                pn[HD:128, qb * QB : (qb + 1) * QB], tmp[:]
                            )
                # ---- batched normalization for pair t --------------------
                rec_all = norm_pool.tile([GH, QB], F32, tag="rec_all")
                nc.vector.reciprocal(rec_all[:], den_all[:])
                scr8 = dram_pool.tile([GH, QB], F32, tag="scr8")
                nc.sync.dma_start(scr8, rec_all[:])
                for qb in range(NQB):
                    for i in range(2):
                        r = 2 * qb + i
                        bcast = norm_pool.tile([128, QB], F32, tag="bcast")
                        rows = slice(i * HD, (i + 1) * HD)
                        nc.sync.dma_start(
                            bcast[rows, :],
                            scr8[r : r + 1, :].broadcast_to([HD, QB]),
                        )
                        nc.vector.tensor_mul(
                            otp[t][rows, qb * QB : (qb + 1) * QB],
                            pn[rows, qb * QB : (qb + 1) * QB],
                            bcast[rows, :],
                        )

        # ---- output projection ------------------------------------------
        with tc.tile_pool(name="out_psum", bufs=2, space="PSUM") as out_psum, \
             tc.tile_pool(name="y", bufs=2) as y_pool:
            for ec in range(NDC):
                ps = out_psum.tile([128, S], F32, tag="yp")
                for mt in range(GM // 128):
                    lhsT = wo_sb[:, mt * D + ec * 128 : mt * D + (ec + 1) * 128]
                    for qb in range(NQB):
                        nc.tensor.matmul(
                            ps[:, qb * QB : (qb + 1) * QB],
                            lhsT,
                            otp[mt][:, qb * QB : (qb + 1) * QB],
                            start=(mt == 0),
                            stop=(mt == GM // 128 - 1),
                        )
                y_sb = y_pool.tile([128, S], F32, tag="y")
                nc.vector.tensor_scalar_add(y_sb[:], ps[:], bo_sb[:, ec : ec + 1])
                nc.sync.dma_start(outT[ec * 128 : (ec + 1) * 128, :], y_sb[:])

    return nc


_NC = None
_last_in_maps = None


def _get_program():
    global _NC
    if _NC is None:
        _NC = _build_program()
    return _NC


def build_in_maps(inputs):
    x = np.asarray(inputs["x"], np.float32)
    Wq, bq = inputs["Wq"], inputs["bq"]
    Wk, bk = inputs["Wk"], inputs["bk"]
    Wv, bv = inputs["Wv"], inputs["bv"]
    Wo, bo = inputs["Wo"], inputs["bo"]
    bf = ml_dtypes.bfloat16
    in_maps = []
    for c in range(8):
        b, g = c // 2, c % 2
        sl = slice(g * GM, (g + 1) * GM)
        wo_slice = np.asarray(Wo, np.float32)[:, sl].T  # [512, 1024]
        # fold bv and half of bo into the output bias
        bo_eff = np.asarray(bo, np.float32) / 2.0 + np.asarray(bv, np.float32)[sl] @ wo_slice
        in_maps.append(
            {
                "xT": np.ascontiguousarray(x[b].T).astype(bf),
                "wq": np.ascontiguousarray(np.asarray(Wq, np.float32)[sl, :].T).astype(bf),
                "wk": np.ascontiguousarray(np.asarray(Wk, np.float32)[sl, :].T).astype(bf),
                "wv": np.ascontiguousarray(np.asarray(Wv, np.float32)[sl, :].T).astype(bf),
                "wo": np.ascontiguousarray(
                    wo_slice.reshape(GM // 128, 128, D).transpose(1, 0, 2).reshape(128, (GM // 128) * D)
                ).astype(bf),
                "bq": np.ascontiguousarray(np.asarray(bq, np.float32)[sl]),
                "bk": np.ascontiguousarray(np.asarray(bk, np.float32)[sl]),
                "bo": np.ascontiguousarray(bo_eff.astype(np.float32)),
            }
        )
    return in_maps


def assemble_output(results):
    out = np.empty((B, S, D), np.float32)
    for b in range(B):
        acc = results[2 * b]["outT"].astype(np.float32) + results[2 * b + 1][
            "outT"
        ].astype(np.float32)
        out[b] = acc.T
    return out


def kernel(x, Wq, bq, Wk, bk, Wv, bv, Wo, bo):
    in_maps = build_in_maps(
        dict(x=x, Wq=Wq, bq=bq, Wk=Wk, bk=bk, Wv=Wv, bv=bv, Wo=Wo, bo=bo)
    )
    global _last_in_maps
    _last_in_maps = in_maps
    nc = _get_program()
    res = run_bass_kernel_spmd(nc, in_maps, core_ids=list(range(8)))
    return assemble_output(res.results)

